# revision 27
# baseline (speedup 1.0000x reference)
"""DistMult edge scoring on 8 Trainium2 NeuronCores.

score[e] = sum_d node_emb[src[e], d] * rel_emb[e, d] * node_emb[dst[e], d]

Strategy (data-parallel over edges, per the sharding hint):
  - Edges (src, dst, rel_emb rows) are sharded evenly across the 8 cores;
    node_emb is replicated to every core's DRAM.
  - Per-edge head/tail rows are fetched with dma_gather (ANT gpsimd ucode).
    Its indices are int16, so edges are binned by (src//32768, dst//32768)
    into 16 bins; each bin gathers from a 32768-row window of the table
    with window-local indices.
  - Bins are padded to multiples of 128 and chopped into chunks of up to
    1024 edges; per chunk: gather head, gather tail, load rel, then
    head*tail*rel on DVE and an add-reduce over the hidden dim.
  - node_emb and rel are converted to fp16 on the host: halves the DMA
    bytes (the fp32 run was DMA-bus-bound at ~336 GB/s aggregate) while
    keeping rel err ~5e-4 (products in fp16, accumulation in fp32).
  - The edge permutation is undone on the host when unsharding.

Self-contained: imports only concourse + numpy; all shapes hardcoded.
"""

import numpy as np

from concourse import bacc, mybir
from concourse.bass_utils import run_bass_kernel_spmd
from concourse.tile import TileContext

N_NODES = 100000
N_EDGES = 150000
D = 512
P = 128
N_CORES = 8
EDGES_PER_CORE = N_EDGES // N_CORES      # 18750
# Signed-int16 gather indices cover a 65536-row window around a base row;
# 2 windows cover the 100000-row table -> 4 (src,dst) bins instead of 16.
WINDOW = 65536
N_RANGES = -(-N_NODES // WINDOW)         # 2
N_BINS = N_RANGES * N_RANGES             # 4
# Base rows chosen so idx = row - base fits int16 and (for window 1) most
# indices are non-negative: the gather ucode strips TRAILING negative
# indices, so each chunk's last slot must have idx >= 0 for both gathers.
BASES = [min(a * WINDOW + 32768, N_NODES - 32768) for a in range(N_RANGES)]
CHUNK_TILES = 8                          # max 128-edge tiles per dma_gather
CHUNK = CHUNK_TILES * P                  # 1024
BUFS = 6
N_QUEUES = 4


def plan_chunks(bin_caps):
    """bin_caps: per-bin padded capacities (multiples of 16; 0 = skip).
    Returns (chunks, j_total, c_total); chunk = (bin_id, n_idx, j0, c0).
    n_idx is a multiple of 16; the tile column count is ceil(n_idx/P)."""
    chunks = []
    j = 0  # tile-column offset into rel/score
    c = 0  # int16 column offset into the index tensors
    for b in range(len(bin_caps)):
        off = 0
        while off < bin_caps[b]:
            n = min(CHUNK, bin_caps[b] - off)
            chunks.append((b, n, j, c))
            j += -(-n // P)
            c += n // 16
            off += n
    return chunks, j, c


def build_program(chunks, j_total, c_total, n_nodes=N_NODES, d=D,
                  bases=BASES, n_ranges=N_RANGES, bufs=BUFS):
    """Build the single-core Bass program (same NEFF runs on all cores)."""
    f32 = mybir.dt.float32
    f16 = mybir.dt.float16
    # (Measured: bufs=2 + a 32KB/partition SWDGE ring ran 631us vs 483us for
    # bufs=3 + the default ring — pool depth matters more than ring depth.)
    nc = bacc.Bacc(None, target_bir_lowering=False, num_swdge_queues=N_QUEUES)
    node_emb = nc.declare_dram_parameter("node_emb", [n_nodes, d], f16, isOutput=False)
    rel = nc.declare_dram_parameter("rel", [P, j_total, d], f16, isOutput=False)
    srci = nc.declare_dram_parameter("srci", [P, c_total], mybir.dt.int16, isOutput=False)
    dsti = nc.declare_dram_parameter("dsti", [P, c_total], mybir.dt.int16, isOutput=False)
    score = nc.declare_dram_parameter("score", [P, j_total], f32, isOutput=True)

    with TileContext(nc) as tc:
        with (
            tc.tile_pool(name="const", bufs=1) as cpool,
            tc.tile_pool(name="emb", bufs=bufs) as epool,
            tc.tile_pool(name="prod", bufs=2) as ppool,
        ):
            src_sb = cpool.tile([P, c_total], mybir.dt.int16, tag="srci")
            dst_sb = cpool.tile([P, c_total], mybir.dt.int16, tag="dsti")
            score_sb = cpool.tile([P, j_total], f32, tag="score")
            # Split the index load so the first chunks' gathers aren't gated
            # on the full index tensors landing.
            c_head = min(chunks[0][1] // 16 + (chunks[1][1] // 16 if len(chunks) > 1 else 0), c_total)
            nc.sync.dma_start(out=src_sb[:, :c_head], in_=srci[:, :c_head])
            nc.sync.dma_start(out=dst_sb[:, :c_head], in_=dsti[:, :c_head])
            if c_head < c_total:
                nc.sync.dma_start(out=src_sb[:, c_head:], in_=srci[:, c_head:])
                nc.sync.dma_start(out=dst_sb[:, c_head:], in_=dsti[:, c_head:])
            j_split = None
            for ci, (b, n_idx, j0, c0) in enumerate(chunks):
                a, bb = divmod(b, n_ranges)
                m = -(-n_idx // P)
                w = n_idx // 16
                head = epool.tile([P, CHUNK_TILES, d], f16, tag="head")
                tail = epool.tile([P, CHUNK_TILES, d], f16, tag="tail")
                relt = epool.tile([P, CHUNK_TILES, d], f16, tag="rel")
                nc.gpsimd.dma_gather(
                    head[:, :m, :],
                    node_emb[bases[a] :, :],
                    src_sb[:, c0 : c0 + w],
                    n_idx,
                    n_idx,
                    d,
                    queue_num=(2 * ci) % N_QUEUES,
                )
                nc.gpsimd.dma_gather(
                    tail[:, :m, :],
                    node_emb[bases[bb] :, :],
                    dst_sb[:, c0 : c0 + w],
                    n_idx,
                    n_idx,
                    d,
                    queue_num=(2 * ci + 1) % N_QUEUES,
                )
                nc.sync.dma_start(out=relt[:, :m, :], in_=rel[:, j0 : j0 + m, :])
                nc.vector.tensor_tensor(
                    out=head[:, :m, :], in0=head[:, :m, :], in1=tail[:, :m, :],
                    op=mybir.AluOpType.mult,
                )
                # fp32 product so the add-reduce runs on fp32 input (the
                # fp16-input reduce measured 2x slower than fp32-input).
                prod = ppool.tile([P, CHUNK_TILES, d], f32, tag="prod")
                nc.vector.tensor_tensor(
                    out=prod[:, :m, :], in0=head[:, :m, :], in1=relt[:, :m, :],
                    op=mybir.AluOpType.mult,
                )
                nc.vector.tensor_reduce(
                    out=score_sb[:, j0 : j0 + m], in_=prod[:, :m, :],
                    axis=mybir.AxisListType.X, op=mybir.AluOpType.add,
                )
                if j_split is None and j0 + m >= j_total // 2:
                    j_split = j0 + m
                    nc.sync.dma_start(
                        out=score[:, :j_split], in_=score_sb[:, :j_split]
                    )
            nc.sync.dma_start(
                out=score[:, j_split:], in_=score_sb[:, j_split:]
            )
    # Run the Bacc compile pipeline (register allocation, event-semaphore
    # wait splitting) — the axon run path does not finalize for us.
    nc.finalize()
    return nc


def shard_and_plan(node_emb, rel_emb, src, dst, n_cores=N_CORES,
                   window=WINDOW, n_ranges=N_RANGES, bases=BASES):
    """Contiguous equal edge shards; per-core binning by (src, dst) window
    pair; permute, and build in_maps + unshard positions.

    The gather ucode strips trailing negative indices, so each chunk's last
    slot must be a pad (idx 0) or an edge whose src AND dst indices are
    non-negative; full chunks swap such an edge into the last slot.

    Returns (chunks, j_total, c_total, in_maps, positions) where positions =
    (pos_core, pos_p, pos_j) per global edge.
    """
    node_emb = np.ascontiguousarray(np.asarray(node_emb, dtype=np.float32).astype(np.float16))
    rel_emb = np.asarray(rel_emb, dtype=np.float32).astype(np.float16)
    src64 = np.asarray(src).astype(np.int64)
    dst64 = np.asarray(dst).astype(np.int64)
    d = node_emb.shape[1]
    n_bins = n_ranges * n_ranges
    n_edges = len(src64)

    assert n_edges % n_cores == 0
    epc = n_edges // n_cores
    bins_g = (src64 // window) * n_ranges + (dst64 // window)
    core_bin_edges = [[None] * n_bins for _ in range(n_cores)]
    counts = np.zeros((n_cores, n_bins), np.int64)
    for c in range(n_cores):
        lo = c * epc
        eb = bins_g[lo : lo + epc]
        order = np.argsort(eb, kind="stable") + lo
        counts[c] = np.bincount(eb, minlength=n_bins)
        start = np.zeros(n_bins + 1, np.int64)
        start[1:] = np.cumsum(counts[c])
        for b in range(n_bins):
            core_bin_edges[c][b] = order[start[b] : start[b + 1]]

    caps = counts.max(axis=0)
    caps = (-(-caps // P)) * P  # pad each bin to a multiple of P (0 stays 0)
    chunks, j_total, c_total = plan_chunks(caps)

    pos_core = np.empty(n_edges, np.int8)
    pos_p = np.empty(n_edges, np.int32)
    pos_j = np.empty(n_edges, np.int32)
    in_maps = []
    for c in range(n_cores):
        src16 = np.zeros((P, c_total), np.int16)
        dst16 = np.zeros((P, c_total), np.int16)
        rel_t = np.zeros((P, j_total, d), np.float16)
        for b in range(n_bins):
            a, bb = divmod(b, n_ranges)
            e_all = core_bin_edges[c][b]
            cnt, cap = len(e_all), int(caps[b])
            # slots[i] = edge id, or -1 for pad. Pads sit at the tail.
            slots = np.full(cap, -1, np.int64)
            slots[:cnt] = e_all
            safe = np.zeros(cap, bool)
            safe[:cnt] = (src64[e_all] >= bases[a]) & (dst64[e_all] >= bases[bb])
            safe[cnt:] = True  # pad idx 0 is non-negative
            off = 0
            while off < cap:  # fix trailing slot of each full chunk
                n_idx = min(CHUNK, cap - off)
                last = off + n_idx - 1
                if not safe[last]:
                    cand = off + np.flatnonzero(safe[off:last])
                    if len(cand):  # swap a safe edge into the last slot
                        k = cand[-1]
                    else:  # move the edge to a pad slot in a later chunk
                        pads = off + n_idx + np.flatnonzero(slots[off + n_idx:] < 0)
                        assert len(pads), "no safe edge and no pad slot"
                        k = pads[0]
                    slots[[last, k]] = slots[[k, last]]
                    safe[[last, k]] = safe[[k, last]]
                off += n_idx
            # fill per-chunk index/rel/position tensors from the slot array
            off = 0
            for bc, n_idx, j0, c0 in chunks:
                if bc != b:
                    continue
                sl = slots[off : off + n_idx]
                off += n_idx
                u = np.arange(n_idx)
                p, j = u % P, j0 + u // P
                real = sl >= 0
                e_chunk = sl[real]
                li_s = np.zeros(n_idx, np.int16)
                li_d = np.zeros(n_idx, np.int16)
                li_s[real] = (src64[e_chunk] - bases[a]).astype(np.int16)
                li_d[real] = (dst64[e_chunk] - bases[bb]).astype(np.int16)
                rel_t[p[real], j[real]] = rel_emb[e_chunk]
                pos_core[e_chunk] = c
                pos_p[e_chunk] = p[real]
                pos_j[e_chunk] = j[real]
                w = n_idx // 16
                src16[:, c0 : c0 + w] = np.tile(li_s.reshape(w, 16).T, (8, 1))
                dst16[:, c0 : c0 + w] = np.tile(li_d.reshape(w, 16).T, (8, 1))
        in_maps.append(
            {"node_emb": node_emb, "rel": rel_t, "srci": src16, "dsti": dst16}
        )
    return chunks, j_total, c_total, in_maps, (pos_core, pos_p, pos_j)


def _unshard(results, positions):
    pos_core, pos_p, pos_j = positions
    out = np.empty(len(pos_core), np.float32)
    for c in range(len(results)):
        m = pos_core == c
        sc = np.asarray(results[c]["score"])
        out[m] = sc[pos_p[m], pos_j[m]]
    return out


def _run(node_emb, rel_emb, src, dst, **spmd_kwargs):
    chunks, j_total, c_total, in_maps, positions = shard_and_plan(
        node_emb, rel_emb, src, dst
    )
    nc = build_program(chunks, j_total, c_total)
    res = run_bass_kernel_spmd(nc, in_maps, list(range(N_CORES)), **spmd_kwargs)
    return _unshard(res.results, positions), res


def kernel(node_emb, rel_emb, src, dst):
    out, _ = _run(node_emb, rel_emb, src, dst)
    return out


def _install_ntff_hook():
    """Provide antenv.axon_hooks (absent on this image) so bass_utils can
    NTFF-profile under axon, and skip the S3 artifact upload."""
    import contextlib
    import ctypes
    import sys
    import types

    from concourse import bass_utils as bu

    bu.upload_artifacts = lambda tmpdir: tmpdir  # no network in container

    if "antenv.axon_hooks" in sys.modules:
        return
    lib = ctypes.CDLL("/opt/axon/libaxon_pjrt.so")
    lib.axon_start_nrt_profile.argtypes = [
        ctypes.POINTER(ctypes.c_int64),
        ctypes.c_size_t,
    ]
    lib.axon_start_nrt_profile.restype = ctypes.c_int64
    lib.axon_stop_nrt_profile.argtypes = [ctypes.c_char_p]
    lib.axon_stop_nrt_profile.restype = ctypes.c_int64

    @contextlib.contextmanager
    def _hook(output_dir, device_ids):
        import jax

        jax.devices()
        if device_ids:
            ids = (ctypes.c_int64 * len(device_ids))(*device_ids)
            rc = lib.axon_start_nrt_profile(ids, len(device_ids))
        else:
            rc = lib.axon_start_nrt_profile(None, 0)
        if rc != 0:
            raise RuntimeError(f"axon_start_nrt_profile rc={rc}")
        try:
            yield
        finally:
            n = lib.axon_stop_nrt_profile(str(output_dir).encode())
            print(f"profile: {n} file(s) written to {output_dir}")

    mod = types.ModuleType("antenv.axon_hooks")
    mod.get_axon_ntff_profile_hook = lambda: _hook
    sys.modules["antenv.axon_hooks"] = mod


def kernel_profiled(node_emb, rel_emb, src, dst, trace_cores=None, tmpdir=None):
    """Like kernel() but also returns exec_time_ns from the NTFF profile."""
    _install_ntff_hook()
    out, res = _run(
        node_emb, rel_emb, src, dst,
        trace=True, trace_cores=trace_cores, tmpdir=tmpdir,
    )
    return out, res.exec_time_ns



# revision 30
# speedup vs baseline: 1.4198x; 1.4198x over previous
"""DistMult edge scoring on 8 Trainium2 NeuronCores.

score[e] = sum_d node_emb[src[e], d] * rel_emb[e, d] * node_emb[dst[e], d]

Strategy (data-parallel over edges, per the sharding hint):
  - Edges (src, dst, rel_emb rows) are sharded evenly across the 8 cores;
    node_emb is replicated to every core's DRAM.
  - Per-edge head/tail rows are fetched with dma_gather (ANT gpsimd ucode).
    Its indices are int16, so edges are binned by (src//32768, dst//32768)
    into 16 bins; each bin gathers from a 32768-row window of the table
    with window-local indices.
  - Bins are padded to multiples of 128 and chopped into chunks of up to
    1024 edges; per chunk: gather head, gather tail, load rel, then
    head*tail*rel on DVE and an add-reduce over the hidden dim.
  - node_emb and rel are converted to fp16 on the host: halves the DMA
    bytes (the fp32 run was DMA-bus-bound at ~336 GB/s aggregate) while
    keeping rel err ~5e-4 (products in fp16, accumulation in fp32).
  - The edge permutation is undone on the host when unsharding.

Self-contained: imports only concourse + numpy; all shapes hardcoded.
"""

import numpy as np

from concourse import bacc, mybir
from concourse.bass_utils import run_bass_kernel_spmd
from concourse.tile import TileContext

N_NODES = 100000
N_EDGES = 150000
D = 512
P = 128
N_CORES = 8
EDGES_PER_CORE = N_EDGES // N_CORES      # 18750
# Signed-int16 gather indices cover a 65536-row window around a base row;
# 2 windows cover the 100000-row table -> 4 (src,dst) bins instead of 16.
WINDOW = 65536
N_RANGES = -(-N_NODES // WINDOW)         # 2
N_BINS = N_RANGES * N_RANGES             # 4
# Base rows chosen so idx = row - base fits int16 and (for window 1) most
# indices are non-negative: the gather ucode strips TRAILING negative
# indices, so each chunk's last slot must have idx >= 0 for both gathers.
BASES = [min(a * WINDOW + 32768, N_NODES - 32768) for a in range(N_RANGES)]
CHUNK_TILES = 8                          # max 128-edge tiles per dma_gather
CHUNK = CHUNK_TILES * P                  # 1024
BUFS = 6
N_QUEUES = 4


def plan_chunks(bin_caps):
    """bin_caps: per-bin padded capacities (multiples of 16; 0 = skip).
    Returns (chunks, j_total, c_total); chunk = (bin_id, n_idx, j0, c0).
    n_idx is a multiple of 16; the tile column count is ceil(n_idx/P)."""
    chunks = []
    j = 0  # tile-column offset into rel/score
    c = 0  # int16 column offset into the index tensors
    for b in range(len(bin_caps)):
        off = 0
        while off < bin_caps[b]:
            n = min(CHUNK, bin_caps[b] - off)
            chunks.append((b, n, j, c))
            j += -(-n // P)
            c += n // 16
            off += n
    return chunks, j, c


def build_program(chunks, j_total, c_total, n_nodes=N_NODES, d=D,
                  bases=BASES, n_ranges=N_RANGES, bufs=BUFS):
    """Build the single-core Bass program (same NEFF runs on all cores)."""
    f32 = mybir.dt.float32
    f16 = mybir.dt.float16
    # (Measured: bufs=2 + a 32KB/partition SWDGE ring ran 631us vs 483us for
    # bufs=3 + the default ring — pool depth matters more than ring depth.)
    nc = bacc.Bacc(None, target_bir_lowering=False, num_swdge_queues=N_QUEUES)
    node_emb = nc.declare_dram_parameter("node_emb", [n_nodes, d], f16, isOutput=False)
    rel = nc.declare_dram_parameter("rel", [P, j_total, d], f16, isOutput=False)
    srci = nc.declare_dram_parameter("srci", [P, c_total], mybir.dt.int16, isOutput=False)
    dsti = nc.declare_dram_parameter("dsti", [P, c_total], mybir.dt.int16, isOutput=False)
    score = nc.declare_dram_parameter("score", [P, j_total], f32, isOutput=True)

    with TileContext(nc) as tc:
        with (
            tc.tile_pool(name="const", bufs=1) as cpool,
            tc.tile_pool(name="emb", bufs=bufs) as epool,
        ):
            src_sb = cpool.tile([P, c_total], mybir.dt.int16, tag="srci")
            dst_sb = cpool.tile([P, c_total], mybir.dt.int16, tag="dsti")
            score_sb = cpool.tile([P, j_total], f32, tag="score")
            # Split the index load so the first chunks' gathers aren't gated
            # on the full index tensors landing.
            c_head = min(chunks[0][1] // 16 + (chunks[1][1] // 16 if len(chunks) > 1 else 0), c_total)
            nc.sync.dma_start(out=src_sb[:, :c_head], in_=srci[:, :c_head])
            nc.sync.dma_start(out=dst_sb[:, :c_head], in_=dsti[:, :c_head])
            if c_head < c_total:
                nc.sync.dma_start(out=src_sb[:, c_head:], in_=srci[:, c_head:])
                nc.sync.dma_start(out=dst_sb[:, c_head:], in_=dsti[:, c_head:])
            j_split = None
            for ci, (b, n_idx, j0, c0) in enumerate(chunks):
                a, bb = divmod(b, n_ranges)
                m = -(-n_idx // P)
                w = n_idx // 16
                head = epool.tile([P, CHUNK_TILES, d], f16, tag="head")
                tail = epool.tile([P, CHUNK_TILES, d], f16, tag="tail")
                relt = epool.tile([P, CHUNK_TILES, d], f16, tag="rel")
                nc.gpsimd.dma_gather(
                    head[:, :m, :],
                    node_emb[bases[a] :, :],
                    src_sb[:, c0 : c0 + w],
                    n_idx,
                    n_idx,
                    d,
                    queue_num=(2 * ci) % N_QUEUES,
                )
                nc.gpsimd.dma_gather(
                    tail[:, :m, :],
                    node_emb[bases[bb] :, :],
                    dst_sb[:, c0 : c0 + w],
                    n_idx,
                    n_idx,
                    d,
                    queue_num=(2 * ci + 1) % N_QUEUES,
                )
                nc.sync.dma_start(out=relt[:, :m, :], in_=rel[:, j0 : j0 + m, :])
                nc.vector.tensor_tensor(
                    out=head[:, :m, :], in0=head[:, :m, :], in1=tail[:, :m, :],
                    op=mybir.AluOpType.mult,
                )
                nc.vector.tensor_tensor(
                    out=head[:, :m, :], in0=head[:, :m, :], in1=relt[:, :m, :],
                    op=mybir.AluOpType.mult,
                )
                # Add-reduce on the otherwise-idle Activation engine (one
                # bypass-activation with accum_out per tile column) so the
                # DVE only runs the two multiplies.
                for j in range(m):
                    nc.scalar.activation(
                        out=head[:, j : j + 1, :],
                        in_=head[:, j : j + 1, :],
                        func=mybir.ActivationFunctionType.Copy,
                        accum_out=score_sb[:, j0 + j : j0 + j + 1],
                    )
                if j_split is None and j0 + m >= j_total // 2:
                    j_split = j0 + m
                    nc.sync.dma_start(
                        out=score[:, :j_split], in_=score_sb[:, :j_split]
                    )
            nc.sync.dma_start(
                out=score[:, j_split:], in_=score_sb[:, j_split:]
            )
    # Run the Bacc compile pipeline (register allocation, event-semaphore
    # wait splitting) — the axon run path does not finalize for us.
    nc.finalize()
    return nc


def shard_and_plan(node_emb, rel_emb, src, dst, n_cores=N_CORES,
                   window=WINDOW, n_ranges=N_RANGES, bases=BASES):
    """Contiguous equal edge shards; per-core binning by (src, dst) window
    pair; permute, and build in_maps + unshard positions.

    The gather ucode strips trailing negative indices, so each chunk's last
    slot must be a pad (idx 0) or an edge whose src AND dst indices are
    non-negative; full chunks swap such an edge into the last slot.

    Returns (chunks, j_total, c_total, in_maps, positions) where positions =
    (pos_core, pos_p, pos_j) per global edge.
    """
    node_emb = np.ascontiguousarray(np.asarray(node_emb, dtype=np.float32).astype(np.float16))
    rel_emb = np.asarray(rel_emb, dtype=np.float32).astype(np.float16)
    src64 = np.asarray(src).astype(np.int64)
    dst64 = np.asarray(dst).astype(np.int64)
    d = node_emb.shape[1]
    n_bins = n_ranges * n_ranges
    n_edges = len(src64)

    assert n_edges % n_cores == 0
    epc = n_edges // n_cores
    bins_g = (src64 // window) * n_ranges + (dst64 // window)
    core_bin_edges = [[None] * n_bins for _ in range(n_cores)]
    counts = np.zeros((n_cores, n_bins), np.int64)
    for c in range(n_cores):
        lo = c * epc
        eb = bins_g[lo : lo + epc]
        order = np.argsort(eb, kind="stable") + lo
        counts[c] = np.bincount(eb, minlength=n_bins)
        start = np.zeros(n_bins + 1, np.int64)
        start[1:] = np.cumsum(counts[c])
        for b in range(n_bins):
            core_bin_edges[c][b] = order[start[b] : start[b + 1]]

    caps = counts.max(axis=0)
    caps = (-(-caps // P)) * P  # pad each bin to a multiple of P (0 stays 0)
    chunks, j_total, c_total = plan_chunks(caps)

    pos_core = np.empty(n_edges, np.int8)
    pos_p = np.empty(n_edges, np.int32)
    pos_j = np.empty(n_edges, np.int32)
    in_maps = []
    for c in range(n_cores):
        src16 = np.zeros((P, c_total), np.int16)
        dst16 = np.zeros((P, c_total), np.int16)
        rel_t = np.zeros((P, j_total, d), np.float16)
        for b in range(n_bins):
            a, bb = divmod(b, n_ranges)
            e_all = core_bin_edges[c][b]
            cnt, cap = len(e_all), int(caps[b])
            # slots[i] = edge id, or -1 for pad. Pads sit at the tail.
            slots = np.full(cap, -1, np.int64)
            slots[:cnt] = e_all
            safe = np.zeros(cap, bool)
            safe[:cnt] = (src64[e_all] >= bases[a]) & (dst64[e_all] >= bases[bb])
            safe[cnt:] = True  # pad idx 0 is non-negative
            off = 0
            while off < cap:  # fix trailing slot of each full chunk
                n_idx = min(CHUNK, cap - off)
                last = off + n_idx - 1
                if not safe[last]:
                    cand = off + np.flatnonzero(safe[off:last])
                    if len(cand):  # swap a safe edge into the last slot
                        k = cand[-1]
                    else:  # move the edge to a pad slot in a later chunk
                        pads = off + n_idx + np.flatnonzero(slots[off + n_idx:] < 0)
                        assert len(pads), "no safe edge and no pad slot"
                        k = pads[0]
                    slots[[last, k]] = slots[[k, last]]
                    safe[[last, k]] = safe[[k, last]]
                off += n_idx
            # fill per-chunk index/rel/position tensors from the slot array
            off = 0
            for bc, n_idx, j0, c0 in chunks:
                if bc != b:
                    continue
                sl = slots[off : off + n_idx]
                off += n_idx
                u = np.arange(n_idx)
                p, j = u % P, j0 + u // P
                real = sl >= 0
                e_chunk = sl[real]
                li_s = np.zeros(n_idx, np.int16)
                li_d = np.zeros(n_idx, np.int16)
                li_s[real] = (src64[e_chunk] - bases[a]).astype(np.int16)
                li_d[real] = (dst64[e_chunk] - bases[bb]).astype(np.int16)
                rel_t[p[real], j[real]] = rel_emb[e_chunk]
                pos_core[e_chunk] = c
                pos_p[e_chunk] = p[real]
                pos_j[e_chunk] = j[real]
                w = n_idx // 16
                src16[:, c0 : c0 + w] = np.tile(li_s.reshape(w, 16).T, (8, 1))
                dst16[:, c0 : c0 + w] = np.tile(li_d.reshape(w, 16).T, (8, 1))
        in_maps.append(
            {"node_emb": node_emb, "rel": rel_t, "srci": src16, "dsti": dst16}
        )
    return chunks, j_total, c_total, in_maps, (pos_core, pos_p, pos_j)


def _unshard(results, positions):
    pos_core, pos_p, pos_j = positions
    out = np.empty(len(pos_core), np.float32)
    for c in range(len(results)):
        m = pos_core == c
        sc = np.asarray(results[c]["score"])
        out[m] = sc[pos_p[m], pos_j[m]]
    return out


def _run(node_emb, rel_emb, src, dst, **spmd_kwargs):
    chunks, j_total, c_total, in_maps, positions = shard_and_plan(
        node_emb, rel_emb, src, dst
    )
    nc = build_program(chunks, j_total, c_total)
    res = run_bass_kernel_spmd(nc, in_maps, list(range(N_CORES)), **spmd_kwargs)
    return _unshard(res.results, positions), res


def kernel(node_emb, rel_emb, src, dst):
    out, _ = _run(node_emb, rel_emb, src, dst)
    return out


def _install_ntff_hook():
    """Provide antenv.axon_hooks (absent on this image) so bass_utils can
    NTFF-profile under axon, and skip the S3 artifact upload."""
    import contextlib
    import ctypes
    import sys
    import types

    from concourse import bass_utils as bu

    bu.upload_artifacts = lambda tmpdir: tmpdir  # no network in container

    if "antenv.axon_hooks" in sys.modules:
        return
    lib = ctypes.CDLL("/opt/axon/libaxon_pjrt.so")
    lib.axon_start_nrt_profile.argtypes = [
        ctypes.POINTER(ctypes.c_int64),
        ctypes.c_size_t,
    ]
    lib.axon_start_nrt_profile.restype = ctypes.c_int64
    lib.axon_stop_nrt_profile.argtypes = [ctypes.c_char_p]
    lib.axon_stop_nrt_profile.restype = ctypes.c_int64

    @contextlib.contextmanager
    def _hook(output_dir, device_ids):
        import jax

        jax.devices()
        if device_ids:
            ids = (ctypes.c_int64 * len(device_ids))(*device_ids)
            rc = lib.axon_start_nrt_profile(ids, len(device_ids))
        else:
            rc = lib.axon_start_nrt_profile(None, 0)
        if rc != 0:
            raise RuntimeError(f"axon_start_nrt_profile rc={rc}")
        try:
            yield
        finally:
            n = lib.axon_stop_nrt_profile(str(output_dir).encode())
            print(f"profile: {n} file(s) written to {output_dir}")

    mod = types.ModuleType("antenv.axon_hooks")
    mod.get_axon_ntff_profile_hook = lambda: _hook
    sys.modules["antenv.axon_hooks"] = mod


def kernel_profiled(node_emb, rel_emb, src, dst, trace_cores=None, tmpdir=None):
    """Like kernel() but also returns exec_time_ns from the NTFF profile."""
    _install_ntff_hook()
    out, res = _run(
        node_emb, rel_emb, src, dst,
        trace=True, trace_cores=trace_cores, tmpdir=tmpdir,
    )
    return out, res.exec_time_ns

